# revision 52
# baseline (speedup 1.0000x reference)
"""Trainium2 Bass kernel for nn_AdaptiveNodeClassifier — V5.

Nodes sharded across 8 cores (dest-owner edge partition). Per layer a bf16
node table z = dinv*(feat@W) sits in DRAM ([100352, HID]); the edge
aggregation bulk-gathers z[src] rows with DMAGatherAnt (<=1024 idxs/call,
int16 bucket-relative indices over 4 x 25088-row buckets) and scatter-adds
them on TensorE: per 128-edge chunk a one-hot (dst%128) matrix multiplies
the gathered rows into a PSUM accumulator per (dst-tile, bucket) group.
Chunks are shared between adjacent groups via -1-masked one-hots, so the
edge stream is nearly unpadded. Self-loops ride along via the PSUM init.

V5 vs V4: the one-hot blocks are generated ON-CHIP (DVE is_equal against
an iota row, from a [128, nseg] bf16 dst%128 table) instead of streaming
67 MB/layer of host-built one-hot matrices from DRAM; activations keep a
single function per phase (casts moved to DVE, PSUM read directly);
injection + log_softmax run batched after the edge phases; layer-2 self
term comes from an SBUF-resident z2 copy.

The per-(bucket, dst-tile) group sizes are maxed across cores so the
instruction stream is SPMD-uniform; shorter cores pad with dead slots
(gather row 0, one-hot column masked off via lo=-1).
"""
from dataclasses import dataclass, field

import numpy as np
import ml_dtypes

import concourse.bacc as bacc
import concourse.bass as bass
import concourse.mybir as mybir
import concourse.tile as tile

F32 = mybir.dt.float32
BF16 = mybir.dt.bfloat16
I16 = mybir.dt.int16

HID = 128
C = 10
LAM = 0.8
N = 100000
NCORES = 8
NPC = 12500
NPP = 12544
NT = NPP // 128          # 98
NBUCKET = 4
# tile-aligned AllGather chunks; chunk k doubles as gather bucket k
# (int16 bucket-relative indices: 8 * CH_R[k] <= 25600 < 32768)
CH_T = [0, 25, 49, 74, 98]                     # chunk tile boundaries
CH_R0 = [t * 128 for t in CH_T[:-1]]           # local row base per chunk
CH_R = [128 * (CH_T[k + 1] - CH_T[k]) for k in range(NBUCKET)]

import os
CALL_CHUNKS = int(os.environ.get("GS_CALLCH", 8))   # 128-edge chunks per gather call
NQUEUES = int(os.environ.get("GS_NQ", 4))           # SWDGE queues to cycle gathers over
GB_BUFS = int(os.environ.get("GS_GBUFS", 3))        # gather buffers in flight per bucket
OH_MODE = os.environ.get("GS_OH", "tt")             # one-hot gen: tt=tensor_tensor/tile, ts=tensor_scalar/seg


@dataclass
class Cfg:
    ncores: int = NCORES
    npc: int = NPC
    # uniform plan, shared by all cores:
    nbt: np.ndarray = None          # [NBUCKET, NT] group slot counts
    buckets: list = field(default_factory=list)
    # per bucket: dict(nchunks, calls=[(chunk0, nch)])
    tiles: list = field(default_factory=list)
    # per tile: (t, seg0, [(b, ch0, nseg, s0, s1)])
    nseg_total: int = 0
    cols_total: int = 0
    maxseg: int = 0
    b1_zero: bool = False
    b2_zero: bool = False


def _make_plan(cfg: Cfg, counts: np.ndarray):
    """counts: [NCORES, NBUCKET, NT] edge counts. Build the uniform plan.

    Segments are numbered tile-major (t outer, b inner) so each tile's
    one-hot blocks are contiguous; gather slots stay bucket-major."""
    cfg.nbt = counts.max(axis=0)                     # [NBUCKET, NT]
    cfg.buckets = []
    cols = 0
    spans = {}                                       # (b, t) -> (ch0, nseg, s0, s1)
    for b in range(NBUCKET):
        sizes = cfg.nbt[b]
        starts = np.concatenate([[0], np.cumsum(sizes)])
        total = int(starts[-1])
        nch = -(-total // 128)
        calls = []
        c0 = 0
        while c0 < nch:
            k = min(CALL_CHUNKS, nch - c0)
            calls.append((c0, k))
            c0 += k
        for t in range(NT):
            n = int(sizes[t])
            if n == 0:
                continue
            s0, s1 = int(starts[t]), int(starts[t] + n)
            ch0, ch1 = s0 // 128, (s1 - 1) // 128
            spans[(b, t)] = (ch0, ch1 - ch0 + 1, s0, s1)
        cfg.buckets.append(dict(nchunks=nch, calls=calls))
        cols += nch * 8                              # slots/16 per chunk
    cfg.tiles = []
    seg = 0
    for t in range(NT):
        seg0 = seg
        parts = []
        for b in range(NBUCKET):
            if (b, t) in spans:
                ch0, nseg, s0, s1 = spans[(b, t)]
                parts.append((b, ch0, nseg, s0, s1))
                seg += nseg
        cfg.tiles.append((t, seg0, parts))
    cfg.nseg_total = seg
    cfg.cols_total = cols
    cfg.maxseg = max((sum(p[2] for p in parts) or 1)
                     for _, _, parts in cfg.tiles)


def build_host_plan(cfg: Cfg, inputs: dict) -> list[dict]:
    x = np.asarray(inputs["x"], np.float32)
    ei = np.asarray(inputs["edge_index"]).astype(np.int64)
    inj = np.asarray(inputs["inject_indices"]).astype(np.int64)
    preds = np.asarray(inputs["initial_preds"], np.float32)

    cfg.b1_zero = not np.any(np.asarray(inputs["b1"]))
    cfg.b2_zero = not np.any(np.asarray(inputs["b2"]))

    src, dst = ei[0], ei[1]
    deg = np.bincount(dst, minlength=N).astype(np.float32) + 1.0
    dinv = (1.0 / np.sqrt(deg)).astype(np.float32)
    mask = np.zeros(N, np.float32)
    mask[np.unique(inj)] = LAM          # pre-scaled: inj = preds * mask

    xs = x * dinv[:, None]

    owner = dst // NPC

    # bucket = AllGather chunk of the source row; table row within bucket b
    # is src_core * CH_R[b] + (src_local - CH_R0[b])
    src_core = src // NPC
    src_loc = src % NPC
    src_bkt = np.searchsorted(np.asarray(CH_R0[1:]), src_loc, side="right")
    src_rel = (src_core * np.asarray(CH_R)[src_bkt]
               + (src_loc - np.asarray(CH_R0)[src_bkt]))

    # per-core edge arrays sorted by (bucket, dst tile, src row) -- src order
    # within a group is free (the lo table captures dst%128 per slot), and
    # ascending gather rows give slightly better HBM locality.
    per_core = []
    counts = np.zeros((NCORES, NBUCKET, NT), np.int64)
    for c in range(NCORES):
        sel = owner == c
        bucket = src_bkt[sel]
        rel = src_rel[sel].astype(np.int64)
        ed = dst[sel] % NPC
        tile_of = ed // 128
        order = np.lexsort((rel, tile_of, bucket))
        bucket, rel, ed, tile_of = (a[order] for a in (bucket, rel, ed, tile_of))
        for b in range(NBUCKET):
            m = bucket == b
            counts[c, b] = np.bincount(tile_of[m], minlength=NT)
        per_core.append((bucket, rel, ed))

    _make_plan(cfg, counts)

    in_maps = []
    for c in range(NCORES):
        bucket, rel, ed = per_core[c]
        gix_cols = []
        los = []
        # per-segment-instance dst%128 (or -1 = dead slot), tile-major
        lo_t = np.full((128, cfg.nseg_total), -1.0, np.float32)
        for b in range(NBUCKET):
            bk = cfg.buckets[b]
            m = bucket == b
            rel_b, ed_b = rel[m], ed[m]
            tile_b = ed_b // 128
            cnt_b = np.bincount(tile_b, minlength=NT)
            # slot layout: group t occupies [starts_u[t], starts_u[t]+cnt_b[t])
            sizes_u = cfg.nbt[b]
            starts_u = np.concatenate([[0], np.cumsum(sizes_u)])
            nslot = bk["nchunks"] * 128
            g = np.zeros(nslot, np.int64)            # pad gather idx -> row 0
            lo = np.full(nslot, -1, np.int64)        # dst%128 or -1
            src_pos = np.concatenate([[0], np.cumsum(cnt_b)])
            for t in range(NT):
                n = int(cnt_b[t])
                if n == 0:
                    continue
                sl = slice(int(starts_u[t]), int(starts_u[t]) + n)
                g[sl] = rel_b[src_pos[t]:src_pos[t] + n]
                lo[sl] = ed_b[src_pos[t]:src_pos[t] + n] % 128
            gix_cols.append(np.tile(
                g.astype(np.int16).reshape(-1, 16).T, (8, 1)))
            los.append(lo)
        # fill the lo table, masking slots that belong to other tiles
        for t, seg0, parts in cfg.tiles:
            sb = seg0
            for b, ch0, nseg, s0u, s1u in parts:
                lo = los[b]
                sl = np.arange(s0u, s1u)
                seg = sb + (sl // 128 - ch0)
                e = sl % 128
                valid = lo[sl] >= 0
                lo_t[e[valid], seg[valid]] = lo[sl][valid]
                sb += nseg
        gixd = np.concatenate(gix_cols, axis=1)

        sl = slice(c * NPC, (c + 1) * NPC)
        # full dinv*x gather table, chunk-block layout (same row mapping
        # as the z2 table): row c2*CH_R[k] + (r - CH_R0[k]) of chunk k
        xtabs = {}
        for k in range(NBUCKET):
            blk = np.zeros((NCORES * CH_R[k], HID), np.float32)
            for c2 in range(NCORES):
                r0, r1 = CH_R0[k], min(CH_R0[k] + CH_R[k], NPC)
                blk[c2 * CH_R[k]:c2 * CH_R[k] + (r1 - r0)] = \
                    xs[c2 * NPC + r0:c2 * NPC + r1]
            xtabs[f"xtab{k}"] = blk.astype(ml_dtypes.bfloat16)
        dv = np.zeros(NPP, np.float32)
        dv[:NPC] = dinv[sl]
        rdv = np.zeros(NPP, np.float32)
        rdv[:NPC] = 1.0 / dinv[sl]
        mk = np.zeros(NPP, np.float32)
        mk[:NPC] = mask[sl]
        pr = np.zeros((NPP, C), np.float32)
        pr[:NPC] = preds[sl]

        xts = np.zeros((128, NPP), np.float32)
        xts[:, :NPC] = xs[sl].T
        iota = np.tile(np.arange(128, dtype=np.float32), (128, 1))
        im = {
            **xtabs,
            "xts": xts.astype(ml_dtypes.bfloat16),
            "dinv": np.ascontiguousarray(dv.reshape(NT, 128).T),
            "rdinv": np.ascontiguousarray(rdv.reshape(NT, 128).T),
            "maskt": np.ascontiguousarray(mk.reshape(NT, 128).T),
            "predst": np.ascontiguousarray(
                pr.reshape(NT, 128, C).transpose(1, 0, 2)).reshape(128, NT * C),
            "gixd": gixd,
            "lot": lo_t.astype(ml_dtypes.bfloat16),
            "iotab": iota.astype(ml_dtypes.bfloat16),
            "identb": np.eye(128, dtype=ml_dtypes.bfloat16),
            "identf": np.eye(128, dtype=np.float32),
            "w1b": np.asarray(inputs["W1"], np.float32).astype(ml_dtypes.bfloat16),
            "w2b": np.asarray(inputs["W2"], np.float32).astype(ml_dtypes.bfloat16),
            "wl1b": np.asarray(inputs["Wl1"], np.float32).astype(ml_dtypes.bfloat16),
            "wl2b": np.asarray(inputs["Wl2"], np.float32).astype(ml_dtypes.bfloat16),
            "wm1h": (0.5 * np.asarray(inputs["Wm1"], np.float32)).astype(
                ml_dtypes.bfloat16),
            "wm2b": np.asarray(inputs["Wm2"], np.float32).astype(ml_dtypes.bfloat16),
            "b1r": np.tile(np.asarray(inputs["b1"], np.float32), (128, 1)),
            "b2r": np.tile(np.asarray(inputs["b2"], np.float32), (128, 1)),
            "bm1c": np.asarray(inputs["bm1"], np.float32).reshape(128, 1),
            "bm2c": np.asarray(inputs["bm2"], np.float32).reshape(C, 1),
        }
        in_maps.append(im)
    return in_maps


def build_graph(cfg: Cfg) -> bacc.Bacc:
    nc = bacc.Bacc("TRN2", target_bir_lowering=False, debug=False,
                   num_devices=cfg.ncores, num_swdge_queues=NQUEUES)

    xts_d = nc.dram_tensor("xts", [128, NPP], BF16, kind="ExternalInput")
    dinv_d = nc.dram_tensor("dinv", [128, NT], F32, kind="ExternalInput")
    rdinv_d = nc.dram_tensor("rdinv", [128, NT], F32, kind="ExternalInput")
    mask_d = nc.dram_tensor("maskt", [128, NT], F32, kind="ExternalInput")
    preds_d = nc.dram_tensor("predst", [128, NT * C], F32, kind="ExternalInput")
    gixd_d = nc.dram_tensor("gixd", [128, cfg.cols_total], I16,
                            kind="ExternalInput")
    lot_d = nc.dram_tensor("lot", [128, cfg.nseg_total], BF16,
                           kind="ExternalInput")
    iotab_d = nc.dram_tensor("iotab", [128, 128], BF16, kind="ExternalInput")
    identb_d = nc.dram_tensor("identb", [128, 128], BF16, kind="ExternalInput")
    identf_d = nc.dram_tensor("identf", [128, 128], F32, kind="ExternalInput")
    wbd = {k: nc.dram_tensor(k, [128, 128], BF16, kind="ExternalInput")
           for k in ["w1b", "w2b", "wl1b", "wl2b", "wm1h"]}
    wm2_d = nc.dram_tensor("wm2b", [128, C], BF16, kind="ExternalInput")
    b1r_d = nc.dram_tensor("b1r", [128, 128], F32, kind="ExternalInput")
    b2r_d = nc.dram_tensor("b2r", [128, 128], F32, kind="ExternalInput")
    bm1c_d = nc.dram_tensor("bm1c", [128, 1], F32, kind="ExternalInput")
    bm2c_d = nc.dram_tensor("bm2c", [C, 1], F32, kind="ExternalInput")
    # out[p, t*C + c] = log_softmax row t*128+p (host reorders)
    out_d = nc.dram_tensor("out", [128, NT * C], F32, kind="ExternalOutput")

    xtab_d = [nc.dram_tensor(f"xtab{k}", [NCORES * CH_R[k], HID], BF16,
                             kind="ExternalInput") for k in range(NBUCKET)]
    z2loc_d = [nc.dram_tensor(f"z2loc{k}", [CH_R[k], HID], BF16)
               for k in range(NBUCKET)]
    ztab2_d = [nc.dram_tensor(f"ztab2_{k}", [NCORES * CH_R[k], HID], BF16,
                              addr_space="Shared") for k in range(NBUCKET)]
    rg = [list(range(cfg.ncores))]

    def chunk_of_tile(t):
        for k in range(NBUCKET):
            if t < CH_T[k + 1]:
                return k
        raise AssertionError(t)

    with tile.TileContext(nc) as tc:
        with (
            tc.tile_pool(name="const", bufs=1) as const,
            tc.tile_pool(name="work", bufs=4) as work,
            tc.tile_pool(name="gp", bufs=4) as gpool,
            tc.tile_pool(name="mh", bufs=2) as mhpool,
            tc.tile_pool(name="psum", bufs=2, space="PSUM") as psum,
            tc.tile_pool(name="psumb", bufs=1, space="PSUM") as psumb,
        ):
            def load_const(dram, shape, dtype=F32):
                t = const.tile(shape, dtype, tag=dram.name, name=f"{dram.name}_sb")
                nc.sync.dma_start(t[:], dram[:])
                return t

            # critical path first: the gather index table (edge-phase-1
            # gathers of the host-staged x table start immediately), bucket
            # 0's columns first
            gix_t = const.tile([128, cfg.cols_total], I16, tag="gixd",
                               name="gixd_sb")
            cbs = [0]
            for b in range(NBUCKET):
                cbs.append(cbs[-1] + cfg.buckets[b]["nchunks"] * 8)
            for b in range(NBUCKET):
                eng = nc.sync if b % 2 == 0 else nc.scalar
                eng.dma_start(gix_t[:, cbs[b]:cbs[b + 1]],
                              gixd_d[:, cbs[b]:cbs[b + 1]])
            lo_t = load_const(lot_d, [128, cfg.nseg_total], BF16)
            iota_t = load_const(iotab_d, [128, 128], BF16)
            identb = load_const(identb_d, [128, 128], BF16)
            xts_t = const.tile([128, NPP], BF16, tag="xts", name="xts_sb")
            for k in range(NBUCKET):
                csl = slice(CH_R0[k], CH_R0[k] + CH_R[k])
                eng = nc.sync if k % 2 == 0 else nc.scalar
                eng.dma_start(xts_t[:, csl], xts_d[:, csl])
            wb = {k2: load_const(d2, [128, 128], BF16)
                  for k2, d2 in wbd.items()}
            dinv_t = load_const(dinv_d, [128, NT])
            rdinv_t = load_const(rdinv_d, [128, NT])
            mask_t = load_const(mask_d, [128, NT])
            preds_t = load_const(preds_d, [128, NT * C])
            identf = load_const(identf_d, [128, 128], F32)
            b1r_t = load_const(b1r_d, [128, 128])
            b2r_t = load_const(b2r_d, [128, 128])
            bm1c_t = load_const(bm1c_d, [128, 1])
            bm2c_t = load_const(bm2c_d, [C, 1])
            wm2b = load_const(wm2_d, [128, C], BF16)

            htb = const.tile([128, NPP], BF16, tag="htb", name="htb")
            z2sb = const.tile([128, NPP], BF16, tag="z2sb", name="z2sb")
            lgall = const.tile([128, NT * C], F32, tag="lgall", name="lgall")

            # ---- edge phase: interleaved per-bucket gather streams (one
            # SWDGE queue each) + tile-major one-hot matmul accumulation,
            # with one-hot blocks generated on DVE and the per-tile
            # post-processing fused in. ----
            def edge_phase(tabs, self_mm, post, fire_pre=None):
                col_base = []
                cb = 0
                for b in range(NBUCKET):
                    col_base.append(cb)
                    cb += cfg.buckets[b]["nchunks"] * 8
                gb_of = [dict() for _ in range(NBUCKET)]
                next_call = [0] * NBUCKET
                emitted = [0] * NBUCKET

                BUFS_B = [GB_BUFS + 2, GB_BUFS + 2, GB_BUFS + 1, GB_BUFS]

                def pump(b, need):
                    while emitted[b] < need:
                        c0, nch = cfg.buckets[b]["calls"][next_call[b]]
                        K = nch * 128
                        gb = gpool.tile([128, CALL_CHUNKS, HID], BF16,
                                        tag=f"gb{b}", name="gb",
                                        bufs=BUFS_B[b])
                        co = col_base[b] + c0 * 8
                        qpb = max(1, NQUEUES // NBUCKET)
                        nc.gpsimd.dma_gather(
                            gb[:, :nch, :], tabs[b],
                            gix_t[:, co:co + K // 16], K, K, HID,
                            queue_num=(b * qpb + next_call[b] % qpb) % NQUEUES)
                        for ch in range(c0, c0 + nch):
                            gb_of[b][ch] = (gb, ch - c0)
                        next_call[b] += 1
                        emitted[b] = c0 + nch

                # prefetch each bucket's stream to buffer depth; earliest
                # buckets first (their tables arrive first)
                if fire_pre is not None:
                    fire_pre()
                for b in range(NBUCKET):
                    pump(b, (BUFS_B[b] - 1) * CALL_CHUNKS)

                pending = None
                for t, seg0, parts in cfg.tiles:
                    nseg_t = sum(p[2] for p in parts)
                    dsl = slice(t * 128, (t + 1) * 128)
                    for b, ch0, nseg, _, _ in parts:
                        pump(b, ch0 + nseg)
                    # one-hot blocks for all of this tile's segments, one
                    # DVE op: oh[p, s, f] = (lo[p, seg0+s] == iota[f])
                    mht = mhpool.tile([128, cfg.maxseg * 128], BF16,
                                      tag="mh", name="mh")
                    oh = mht[:, :nseg_t * 128]
                    oh3 = bass.AP(oh.tensor, oh.offset,
                                  [oh.ap[0], [128, nseg_t], [1, 128]])
                    lo3 = lo_t[:, seg0:seg0 + nseg_t].to_broadcast(
                        [128, nseg_t, 128])
                    io = iota_t[:, :]
                    io3 = bass.AP(io.tensor, io.offset,
                                  [io.ap[0], [0, nseg_t], [1, 128]])
                    nc.vector.tensor_tensor(oh3, lo3, io3,
                                            mybir.AluOpType.is_equal)
                    ps = psum.tile([128, 128], F32, tag="acc", name="eps",
                                   bufs=4)
                    self_mm(ps, dsl, stop=(nseg_t == 0))
                    k = 0
                    for b, ch0, nseg, _, _ in parts:
                        for j in range(nseg):
                            gbt, ci = gb_of[b][ch0 + j]
                            nc.tensor.matmul(
                                ps[:], gbt[:, ci, :],
                                mht[:, k * 128:(k + 1) * 128],
                                start=False, stop=(k == nseg_t - 1))
                            k += 1
                    # defer the post by one tile: its cross-engine waits
                    # (PSUM->SBUF copy, DVE chain) overlap the next tile's
                    # segment matmuls instead of stalling TensorE in-order
                    if pending is not None:
                        post(*pending)
                    pending = (t, dsl, ps)
                if pending is not None:
                    post(*pending)

            # layer 1 self term: acc_T init = (dinv*x)_own^T = xts slice
            def self1(ps, dsl, stop):
                nc.tensor.matmul(ps[:], identb[:], xts_t[:, dsl],
                                 start=True, stop=stop)

            # fused P1: h = relu(0.5*(low1 + x@Wl1)); z2 own rows to DRAM
            def post1(t, dsl, ps):
                aT = work.tile([128, 128], BF16, tag="aT", name="aT")
                nc.vector.tensor_copy(aT[:], ps[:])
                psw = psum.tile([128, 128], F32, tag="ps", name="psw",
                                bufs=1)
                nc.tensor.matmul(psw[:], aT[:], wb["w1b"][:])
                u = work.tile([128, 128], F32, tag="u", name="u")
                nc.vector.tensor_scalar(u[:], psw[:],
                                        dinv_t[:, t:t + 1], None,
                                        mybir.AluOpType.mult)
                if cfg.b1_zero:
                    u2 = u
                else:
                    u2 = work.tile([128, 128], F32, tag="u2", name="u2")
                    nc.vector.tensor_tensor(u2[:], u[:], b1r_t[:],
                                            mybir.AluOpType.add)
                hp = psum.tile([128, 128], F32, tag="ps", name="hp", bufs=1)
                nc.tensor.matmul(hp[:], xts_t[:, dsl], wb["wl1b"][:])
                hpw = work.tile([128, 128], F32, tag="hpw", name="hpw")
                nc.vector.tensor_scalar(hpw[:], hp[:],
                                        rdinv_t[:, t:t + 1], None,
                                        mybir.AluOpType.mult)
                w = work.tile([128, 128], F32, tag="w", name="w")
                nc.vector.tensor_tensor(w[:], u2[:], hpw[:], mybir.AluOpType.add)
                hb = work.tile([128, 128], BF16, tag="hb", name="hb")
                nc.scalar.activation(hb[:], w[:],
                                     mybir.ActivationFunctionType.Relu,
                                     scale=0.5)
                pt = psumb.tile([128, 128], BF16, tag="ptb", name="ptb")
                nc.tensor.transpose(pt[:], hb[:], identb[:])
                nc.vector.tensor_copy(htb[:, dsl], pt[:])
                nc.vector.tensor_scalar(z2sb[:, dsl], hb[:],
                                        dinv_t[:, t:t + 1], None,
                                        mybir.AluOpType.mult)
                k = chunk_of_tile(t)
                lsl = slice(t * 128 - CH_R0[k], (t + 1) * 128 - CH_R0[k])
                nc.sync.dma_start(z2loc_d[k][lsl, :], z2sb[:, dsl])
                if t + 1 == CH_T[k + 1] and k < NBUCKET - 1:
                    nc.gpsimd.collective_compute(
                        "AllGather", mybir.AluOpType.bypass,
                        ins=[z2loc_d[k][:]], outs=[ztab2_d[k][:]],
                        replica_groups=rg)

            edge_phase([xtab_d[b][:, :] for b in range(NBUCKET)],
                       self1, post1)

            # layer 2 self term: acc_T init = (dinv*h)_own^T
            def self2(ps, dsl, stop):
                nc.tensor.matmul(ps[:], z2sb[:, dsl], identb[:],
                                 start=True, stop=stop)

            # fused P2: h2, MLP head -> logits into lgall (tail after phase)
            def post2(t, dsl, ps):
                aT = work.tile([128, 128], BF16, tag="aT", name="aT2")
                nc.vector.tensor_copy(aT[:], ps[:])
                psw = psum.tile([128, 128], F32, tag="ps", name="psw2",
                                bufs=1)
                nc.tensor.matmul(psw[:], aT[:], wb["w2b"][:])
                u = work.tile([128, 128], F32, tag="u", name="u_2")
                nc.vector.tensor_scalar(u[:], psw[:],
                                        dinv_t[:, t:t + 1], None,
                                        mybir.AluOpType.mult)
                if cfg.b2_zero:
                    u2 = u
                else:
                    u2 = work.tile([128, 128], F32, tag="u2", name="u2_2")
                    nc.vector.tensor_tensor(u2[:], u[:], b2r_t[:],
                                            mybir.AluOpType.add)
                hp = psum.tile([128, 128], F32, tag="ps", name="hp2", bufs=1)
                nc.tensor.matmul(hp[:], htb[:, dsl], wb["wl2b"][:])
                w = work.tile([128, 128], BF16, tag="h2p", name="h2p")
                nc.vector.tensor_tensor(w[:], u2[:], hp[:], mybir.AluOpType.add)
                pt = psumb.tile([128, 128], BF16, tag="ptb", name="ptb2")
                nc.tensor.transpose(pt[:], w[:], identb[:])
                h2pt = work.tile([128, 128], BF16, tag="h2pt", name="h2pt")
                nc.vector.tensor_copy(h2pt[:], pt[:])
                t1p = psum.tile([128, 128], F32, tag="hd", name="t1p", bufs=2)
                nc.tensor.matmul(t1p[:], wb["wm1h"][:], h2pt[:])
                t1t = work.tile([128, 128], BF16, tag="t1t", name="t1t")
                nc.scalar.activation(t1t[:], t1p[:],
                                     mybir.ActivationFunctionType.Relu,
                                     bias=bm1c_t[:])
                lgp = psum.tile([C, 128], F32, tag="hd", name="lgp", bufs=2)
                nc.tensor.matmul(lgp[:], wm2b[:], t1t[:])
                lgt = work.tile([C, 128], F32, tag="lgt", name="lgt")
                nc.vector.tensor_scalar(lgt[:], lgp[:], bm2c_t[:], None,
                                        mybir.AluOpType.add)
                ptl = psum.tile([128, C], F32, tag="hd", name="ptl", bufs=2)
                nc.tensor.transpose(ptl[:], lgt[:], identf[:C, :C])
                nc.vector.tensor_copy(lgall[:, t * C:(t + 1) * C], ptl[:])

            def fire2():
                nc.gpsimd.collective_compute(
                    "AllGather", mybir.AluOpType.bypass,
                    ins=[z2loc_d[NBUCKET - 1][:]],
                    outs=[ztab2_d[NBUCKET - 1][:]], replica_groups=rg)

            edge_phase([ztab2_d[b][:, :] for b in range(NBUCKET)],
                       self2, post2, fire_pre=fire2)

            # ---- batched tail: injection + log_softmax over [128, NT*C] ----
            def b3(ap, inner):
                # [128, NT] per-tile values broadcast across the C classes
                return bass.AP(ap.tensor, ap.offset,
                               [ap.ap[0], [1, NT], [0, inner]])

            def g3(ap):
                # [128, NT*C] viewed as [128, NT, C]
                return bass.AP(ap.tensor, ap.offset,
                               [ap.ap[0], [C, NT], [1, C]])

            def r3(ap):
                # [128, NT] viewed as [128, NT, 1] (reduce output)
                return bass.AP(ap.tensor, ap.offset,
                               [ap.ap[0], [1, NT], [1, 1]])

            inj = work.tile([128, NT * C], F32, tag="tail", name="inj",
                            bufs=1)
            nc.vector.tensor_tensor(g3(inj[:]), g3(preds_t[:]),
                                    b3(mask_t[:], C), mybir.AluOpType.mult)
            lg = work.tile([128, NT * C], F32, tag="tail2", name="lg", bufs=1)
            nc.vector.tensor_tensor(lg[:], lgall[:], inj[:],
                                    mybir.AluOpType.add)
            mneg = work.tile([128, NT], F32, tag="tail3", name="mneg", bufs=1)
            nc.vector.tensor_reduce(r3(mneg[:]), g3(lg[:]),
                                    mybir.AxisListType.X,
                                    mybir.AluOpType.max, negate=True)
            lgm = work.tile([128, NT * C], F32, tag="tail4", name="lgm",
                            bufs=1)
            nc.vector.tensor_tensor(g3(lgm[:]), g3(lg[:]), b3(mneg[:], C),
                                    mybir.AluOpType.add)
            e = work.tile([128, NT * C], F32, tag="tail", name="e", bufs=1)
            nc.scalar.activation(e[:], lgm[:],
                                 mybir.ActivationFunctionType.Exp)
            s = work.tile([128, NT], F32, tag="tail3", name="s", bufs=1)
            nc.vector.tensor_reduce(r3(s[:]), g3(e[:]),
                                    mybir.AxisListType.X,
                                    mybir.AluOpType.add)
            ls = work.tile([128, NT], F32, tag="tail5", name="ls", bufs=1)
            nc.scalar.activation(ls[:], s[:],
                                 mybir.ActivationFunctionType.Ln)
            o = work.tile([128, NT * C], F32, tag="tail2", name="o", bufs=1)
            nc.vector.tensor_tensor(g3(o[:]), g3(lgm[:]), b3(ls[:], C),
                                    mybir.AluOpType.subtract)
            nc.sync.dma_start(out_d[:], o[:])

    nc.compile()
    return nc


def kernel(**inputs) -> np.ndarray:
    from concourse.bass_utils import run_bass_kernel_spmd

    cfg = Cfg()
    in_maps = build_host_plan(cfg, inputs)
    nc = build_graph(cfg)
    res = run_bass_kernel_spmd(nc, in_maps, core_ids=list(range(cfg.ncores)))
    return assemble(cfg, [res.results[c]["out"] for c in range(cfg.ncores)])


def assemble(cfg, outs) -> np.ndarray:
    # outs[c] is [128, NT*C]: row t*128+p of core c lives at [p, t*C:(t+1)*C]
    full = []
    for c in range(cfg.ncores):
        a = outs[c].reshape(128, NT, C).transpose(1, 0, 2).reshape(NPP, C)
        full.append(a[:NPC])
    return np.concatenate(full, 0)
